# revision 1
# baseline (speedup 1.0000x reference)
"""Trainium2 Bass kernel for nn_Caps_Layer (capsule routing layer).

Full inputs: x [64, 512, 768] f32, W [1, 768, 1024] f32.
Output: [64, 16, 64] f32.

Sharding: pure data parallel over batch — 8 elems per core on 8 cores; W
replicated. Per batch element:
  u = x[b] @ W            # [512, 1024], col m = i*64 + k (capsule i, dim k)
  3 routing iters: c = softmax_i(b_log); v_i = sum_j c_ij u_ij; outn = l2norm(v);
                   b_log_ij = <outn_i, u_ij>
All matmuls in float32r (full PE rate, ~1e-4 rel err). x is pre-transposed on
the host so the pass-A lhsT tiles DMA straight in. u is kept in two on-chip
layouts (u_flat [j, m] and PE-transposed u_flatT [m, j]) because the two
routing einsums contract different axes. The iter-0 softmax of zeros is
uniform, and the squash normalization kills scale, so iter 0 uses an all-ones
cT constant instead of a softmax.
"""
import numpy as np
import concourse.mybir as mybir
import concourse.tile as tile
from concourse import bacc
from concourse.bass_utils import run_bass_kernel_spmd

F32 = mybir.dt.float32
F32R = mybir.dt.float32r
AX = mybir.AxisListType
ALU = mybir.AluOpType
ACT_F = mybir.ActivationFunctionType

N_CORES = 8
B_FULL = 64
B_LOCAL = B_FULL // N_CORES  # 8
IN_J = 512
DF = 768
NCAP = 16
DCAP = 64
M = NCAP * DCAP  # 1024
T_EPS = 1e-7
ROUTINGS = 3
JT = IN_J // 128  # 4
KT = DF // 128  # 6
MT = M // 128  # 8


def _make_consts():
    ident = np.eye(128, dtype=np.float32)
    ones16 = np.ones((128, NCAP), dtype=np.float32)
    mask16 = np.zeros((NCAP, M), dtype=np.float32)
    for i in range(NCAP):
        mask16[i, i * DCAP : (i + 1) * DCAP] = 1.0
    return {"ident": ident, "ones16": ones16, "mask16": mask16}


def build_nc(b_local: int = B_LOCAL, reps: int = 1):
    """Per-core kernel. x arrives pre-transposed: [b_local, 768, 512]."""
    nc = bacc.Bacc(None, target_bir_lowering=False, debug=True)

    x_d = nc.declare_dram_parameter("x", [b_local, DF, IN_J], F32R, isOutput=False)
    w_d = nc.declare_dram_parameter("W", [DF, M], F32R, isOutput=False)
    id_d = nc.declare_dram_parameter("ident", [128, 128], F32R, isOutput=False)
    on_d = nc.declare_dram_parameter("ones16", [128, NCAP], F32R, isOutput=False)
    mk_d = nc.declare_dram_parameter("mask16", [NCAP, M], F32, isOutput=False)
    out_d = nc.declare_dram_parameter(
        "out", [b_local, NCAP, DCAP], F32, isOutput=True
    )

    with tile.TileContext(nc) as tc:
        with (
            tc.tile_pool(name="const", bufs=1) as const,
            tc.tile_pool(name="xp", bufs=2) as xp,
            tc.tile_pool(name="up", bufs=2) as up,
            tc.tile_pool(name="rs", bufs=4) as rs,
            tc.tile_pool(name="pbig", bufs=3, space="PSUM") as pbig,
            tc.tile_pool(name="pout2", bufs=1, space="PSUM") as pout2,
            tc.tile_pool(name="pmisc", bufs=2, space="PSUM") as pmisc,
        ):
            ident = const.tile([128, 128], F32R)
            nc.sync.dma_start(out=ident, in_=id_d[:, :])
            ones16 = const.tile([128, NCAP], F32R)
            nc.sync.dma_start(out=ones16, in_=on_d[:, :])
            mask16 = const.tile([NCAP, M], F32)
            nc.sync.dma_start(out=mask16, in_=mk_d[:, :])
            eps_t = const.tile([128, 1], F32)
            nc.vector.memset(eps_t, T_EPS)
            w_t = const.tile([128, KT, M], F32R)
            nc.sync.dma_start(
                out=w_t, in_=w_d.rearrange("(kt p) m -> p kt m", p=128)
            )

            for _rep in range(reps):
              for b in range(b_local):
                # xT [f part (6x128), j free (512)] straight from HBM
                xT = xp.tile([128, KT, IN_J], F32R, tag="xT")
                nc.sync.dma_start(
                    out=xT, in_=x_d[b].rearrange("(kt p) j -> p kt j", p=128)
                )

                # pass A: u_flat [j part (4x128), m (1024)]
                u_f = up.tile([128, JT, M], F32R, tag="u")
                for jt in range(JT):
                    for nh in range(2):
                        acc = pbig.tile([128, 512], F32, tag="pbig")
                        for kt in range(KT):
                            nc.tensor.matmul(
                                acc,
                                xT[:, kt, jt * 128 : (jt + 1) * 128],
                                w_t[:, kt, nh * 512 : (nh + 1) * 512],
                                start=(kt == 0),
                                stop=(kt == KT - 1),
                            )
                        nc.vector.tensor_copy(
                            u_f[:, jt, nh * 512 : (nh + 1) * 512], acc
                        )

                # u_flatT [m part (8x128), j (512)] via PE transpose
                uT = up.tile([128, MT, IN_J], F32R, tag="uT")
                for mt in range(MT):
                    tp2 = pbig.tile([128, 512], F32R, tag="pbig")
                    for jt in range(JT):
                        nc.tensor.transpose(
                            tp2[:, jt * 128 : (jt + 1) * 128],
                            u_f[:, jt, mt * 128 : (mt + 1) * 128],
                            ident,
                        )
                    nc.vector.tensor_copy(uT[:, mt, :], tp2)

                # routing
                cT = None  # [128, JT, 16] f32r; None => uniform iter 0
                for it in range(ROUTINGS):
                    # out2 [16, 1024] = sum_jt cT_jt.T @ u_flat_jt
                    o2 = pout2.tile([NCAP, 2, 512], F32, tag="o2")
                    for nh in range(2):
                        for jt in range(JT):
                            lhs = ones16 if cT is None else cT[:, jt, :]
                            nc.tensor.matmul(
                                o2[:, nh, :],
                                lhs,
                                u_f[:, jt, nh * 512 : (nh + 1) * 512],
                                start=(jt == 0),
                                stop=(jt == JT - 1),
                            )
                    # m1 = out2 * blockdiag-mask  (diag blocks = v)
                    m1 = rs.tile([NCAP, M], F32, tag="m1")
                    nc.vector.tensor_tensor(
                        out=m1,
                        in0=o2.rearrange("p a b -> p (a b)"),
                        in1=mask16,
                        op=ALU.mult,
                    )
                    # sumsq + 1/sqrt(.+eps)   (NOT accum_out: it faults HW)
                    sq = rs.tile([NCAP, M], F32, tag="sq")
                    nc.scalar.activation(out=sq, in_=m1, func=ACT_F.Square)
                    s_sq = rs.tile([NCAP, 1], F32, tag="ssq")
                    nc.vector.tensor_reduce(
                        out=s_sq, in_=sq, op=ALU.add, axis=AX.X
                    )
                    sd = rs.tile([NCAP, 1], F32, tag="sd")
                    nc.scalar.activation(
                        out=sd, in_=s_sq, func=ACT_F.Sqrt, bias=eps_t[:NCAP, :]
                    )
                    rsn = rs.tile([NCAP, 1], F32, tag="rsn")
                    nc.vector.reciprocal(out=rsn, in_=sd)

                    if it == ROUTINGS - 1:
                        # final: fold i-axis (zeros off-diag) then scale
                        red = rs.tile([NCAP, DCAP], F32, tag="red")
                        nc.vector.tensor_reduce(
                            out=red,
                            in_=m1.rearrange("p (i k) -> p k i", i=NCAP),
                            op=ALU.add,
                            axis=AX.X,
                        )
                        ob = rs.tile([NCAP, DCAP], F32, tag="ob")
                        nc.vector.tensor_scalar_mul(ob, red, rsn)
                        nc.sync.dma_start(out=out_d[b], in_=ob)
                        break

                    # ODT [16, 1024] = m1 * rsn (f32r, feeds PE transpose)
                    odt = rs.tile([NCAP, M], F32R, tag="odt")
                    nc.vector.tensor_scalar_mul(odt, m1, rsn)

                    # OD [m part, 16] via 8 PE transposes of [16, 128]
                    pod = pmisc.tile([128, MT, NCAP], F32R, tag="pm")
                    for mt in range(MT):
                        nc.tensor.transpose(
                            pod[:, mt, :],
                            odt[:, mt * 128 : (mt + 1) * 128],
                            ident[:NCAP, :NCAP],
                        )
                    od = rs.tile([128, MT, NCAP], F32R, tag="od")
                    nc.vector.tensor_copy(od, pod)

                    # b_new [16, 512] = sum_mt OD_mt.T @ uT_mt
                    pb = pmisc.tile([NCAP, 512], F32, tag="pm")
                    for mt in range(MT):
                        nc.tensor.matmul(
                            pb,
                            od[:, mt, :],
                            uT[:, mt, :],
                            start=(mt == 0),
                            stop=(mt == MT - 1),
                        )
                    bsb = rs.tile([NCAP, 512], F32R, tag="bsb")
                    nc.vector.tensor_copy(bsb, pb)

                    # bT [j part (4x128), 16] via 4 PE transposes
                    pbt = pmisc.tile([128, JT, NCAP], F32R, tag="pm")
                    for jt in range(JT):
                        nc.tensor.transpose(
                            pbt[:, jt, :],
                            bsb[:, jt * 128 : (jt + 1) * 128],
                            ident[:NCAP, :NCAP],
                        )
                    bT = rs.tile([128, JT, NCAP], F32, tag="bT")
                    nc.vector.tensor_copy(bT, pbt)

                    # softmax over the 16 capsules (free dim)
                    ex = rs.tile([128, JT, NCAP], F32, tag="ex")
                    nc.scalar.activation(out=ex, in_=bT, func=ACT_F.Exp)
                    esum = rs.tile([128, JT], F32, tag="esum")
                    nc.vector.tensor_reduce(
                        out=esum, in_=ex, op=ALU.add, axis=AX.X
                    )
                    esr = rs.tile([128, JT], F32, tag="esr")
                    nc.vector.reciprocal(out=esr, in_=esum)
                    cT = rs.tile([128, JT, NCAP], F32R, tag="cT")
                    for jt in range(JT):
                        nc.vector.tensor_scalar_mul(
                            cT[:, jt, :], ex[:, jt, :], esr[:, jt : jt + 1]
                        )
    nc.compile()
    return nc


_NC_CACHE = {}


def _get_nc(reps: int = 1):
    if reps not in _NC_CACHE:
        _NC_CACHE[reps] = build_nc(B_LOCAL, reps)
    return _NC_CACHE[reps]


def kernel(x: np.ndarray, W: np.ndarray) -> np.ndarray:
    assert x.shape == (B_FULL, IN_J, DF), x.shape
    W2 = np.ascontiguousarray(np.asarray(W, dtype=np.float32).reshape(DF, M))
    xT = np.ascontiguousarray(
        np.asarray(x, dtype=np.float32).transpose(0, 2, 1)
    )  # [64, 768, 512]
    consts = _make_consts()
    nc = _get_nc()
    in_maps = [
        {"x": xT[c * B_LOCAL : (c + 1) * B_LOCAL], "W": W2, **consts}
        for c in range(N_CORES)
    ]
    res = run_bass_kernel_spmd(nc, in_maps, list(range(N_CORES)))
    out = np.concatenate(
        [res.results[c]["out"] for c in range(N_CORES)], axis=0
    ).astype(np.float32)
    return out
